# revision 1
# baseline (speedup 1.0000x reference)
"""MobileMQA Trainium2 kernel (8 NeuronCores, SPMD).

Reference computation (per batch b of 2):
  q  = x @ wq + bq                         [1024 tok, 512]
  kv = x @ wkv + bkv                       [1024 tok, 1024]
  kv = depthwise3x3_s2_same(kv) + dw_bias  [256 sp, 1024]
  k, v = split(kv)  -> reshape to shared-KV length M=2048 (channel fold)
  attn = softmax(q @ k^T * 0.125); out = attn @ v
  y = out @ wo + bo

Sharding: core c handles batch b=c//4, query chunk j=c%4 (256 tokens).
KV path (proj+conv) is replicated across the 4 cores of a batch (cheap: MQA).

Per-core dataflow (all channel-major / transposed layouts, fp32r matmuls):
  kv^T = wkv^T @ x^T              (PE, f32r)         [1024 ch, 1024 tok]
  conv: strided-window MACs       (DVE+GPSIMD, f32)  [1024 ch, 256 sp]
        bias folded into a host-precomputed bias plane (SAME-padding aware)
  kT2  [128, 2048]: k^T duplicated on both partition halves (row-tiled scores)
  V_aug[mt] [128, 65]: V in [m,d] layout via PE transpose + ones column (z trick)
  qT2  [128, 2048]: q^T per head, duplicated halves
  scores: S^T[mt] = kT2 x qT2     (PE f32r, 2-way row tiling, K=64 pairs)
  expS = exp(S^T * 0.125)         (ACT, out f32r)
  avT += V_aug^T @ expS           (PE f32r, PSUM accum over 16 m-tiles)
        row 64 of avT = softmax denominator z
  normalize: attnT = avT * (1/z)  (DVE stt, z broadcast via GPSIMD)
  y^T = wo^T @ attnT + bo         (PE f32r)          [512, 256]
"""
import sys

for _p in ("/opt/trn_rl_repo", "/opt/trn_rl_repo/concourse"):
    if _p not in sys.path:
        sys.path.insert(0, _p)

import numpy as np

import concourse.bass as bass
import concourse.mybir as mybir
import concourse.tile as tile
from concourse import bacc
from concourse.bass_utils import run_bass_kernel_spmd
from concourse.masks import make_identity

F32 = mybir.dt.float32
F32R = mybir.dt.float32r
AF = mybir.ActivationFunctionType
ALU = mybir.AluOpType

DIM = 512
NH = 8
HD = 64
B, H, W = 2, 32, 32
L = H * W            # 1024 tokens per batch
KH = KW = 16
NS = KH * KW         # 256 conv-output spatial positions
M = NS * NH          # 2048 shared-KV positions
CH = 2 * DIM         # 1024 kv channels
SCALE = HD ** -0.5   # 0.125
PADW = 33            # padded conv input row (32 + 1 SAME-pad)
NPAD = PADW * PADW   # 1089

_NC_CACHE = {}


def _round_f32r(a: np.ndarray) -> np.ndarray:
    """Round fp32 to the fp32r grid (11-bit mantissa, round-to-nearest)."""
    bits = np.ascontiguousarray(a, np.float32).view(np.uint32)
    bits = (bits + np.uint32(0x800)) & np.uint32(0xFFFFF000)
    return bits.view(np.float32)


def _build_program():
    nc = bacc.Bacc(None)

    xT_d = nc.dram_tensor("xT", [DIM, L], F32R, kind="ExternalInput")
    xTc_d = nc.dram_tensor("xTc", [DIM, 256], F32R, kind="ExternalInput")
    wkv_d = nc.dram_tensor("wkv", [DIM, CH], F32R, kind="ExternalInput")
    wq_d = nc.dram_tensor("wq", [DIM, DIM], F32R, kind="ExternalInput")
    wo_d = nc.dram_tensor("wo", [DIM, DIM], F32R, kind="ExternalInput")
    bpl_d = nc.dram_tensor("bpl", [CH, NS], F32, kind="ExternalInput")
    # consts: cols 0-3 bq tiles, 4-75 dw weights (8 ch-tiles x 9 taps), 76-79 bo tiles
    cst_d = nc.dram_tensor("cst", [128, 80], F32, kind="ExternalInput")
    y_d = nc.dram_tensor("y", [DIM, 256], F32, kind="ExternalOutput")

    with tile.TileContext(nc) as tc:
        with tc.tile_pool(name="wp", bufs=1) as wp, \
             tc.tile_pool(name="expp", bufs=6) as expp, \
             tc.tile_pool(name="kvsbp", bufs=3) as kvsbp, \
             tc.tile_pool(name="caccp", bufs=2) as caccp, \
             tc.tile_pool(name="zrbp", bufs=2) as zrbp:

            # ---------------- persistent SBUF + input DMAs ----------------
            # Few large DMAs (HWDGE descriptor time is ~0.6us per dma_start):
            # multi-k-tile SBUF layouts [128, k, n]; kv-proj inputs first.
            cst = wp.tile([128, 80], F32, tag="cst")
            nc.sync.dma_start(out=cst, in_=cst_d[:, :])
            xT = wp.tile([128, 4, L], F32R, tag="xT")
            wkv = wp.tile([128, 4, CH], F32R, tag="wkv")
            bpl = wp.tile([128, 8, NS], F32, tag="bpl")
            xT_r = xT_d[:, :].rearrange("(k p) t -> p k t", p=128)
            wkv_r = wkv_d[:, :].rearrange("(k p) c -> p k c", p=128)
            bpl_r = bpl_d[:, :].rearrange("(t p) s -> p t s", p=128)

            def load_kv_col(c):
                nc.sync.dma_start(out=wkv[:, :, c * 128:(c + 1) * 128],
                                  in_=wkv_r[:, :, c * 128:(c + 1) * 128])

            # kv ch-tiles are consumed in order k0 v0 k1 v1 ... = 0,4,1,5,...
            # xT arrives per k-slice, interleaved with wkv column-0 blocks, so
            # the first kv matmul starts as soon as slice 0 lands.
            for k in range(4):
                nc.sync.dma_start(out=xT[:, k, 0:512], in_=xT_r[:, k, 0:512])
                nc.sync.dma_start(out=wkv[:, k, 0:128], in_=wkv_r[:, k, 0:128])
            for k in range(4):
                nc.sync.dma_start(out=xT[:, k, 512:L], in_=xT_r[:, k, 512:L])
            load_kv_col(4)
            nc.sync.dma_start(out=bpl[:, 0:1, :], in_=bpl_r[:, 0:1, :])
            nc.sync.dma_start(out=bpl[:, 4:5, :], in_=bpl_r[:, 4:5, :])
            wq = wp.tile([128, 4, DIM], F32R, tag="wq")
            nc.sync.dma_start(out=wq,
                              in_=wq_d[:, :].rearrange("(k p) c -> p k c", p=128))
            xTc = wp.tile([128, 4, 256], F32R, tag="xTc")
            nc.sync.dma_start(out=xTc,
                              in_=xTc_d[:, :].rearrange("(k p) t -> p k t", p=128))
            for c in (1, 5, 2, 6, 3, 7):
                load_kv_col(c)
            nc.sync.dma_start(out=bpl[:, 1:4, :], in_=bpl_r[:, 1:4, :])
            nc.sync.dma_start(out=bpl[:, 5:8, :], in_=bpl_r[:, 5:8, :])
            wo = wp.tile([128, 4, DIM], F32R, tag="wo")
            nc.sync.dma_start(out=wo,
                              in_=wo_d[:, :].rearrange("(k p) c -> p k c", p=128))

            ident = wp.tile([128, 128], F32, tag="ident")
            make_identity(nc, ident)
            # preload the exp ACT table during the DMA window
            warm = wp.tile([1, 1], F32, tag="warm")
            nc.vector.memset(warm, 0.0)
            nc.scalar.activation(warm[:, :], warm[:, :], AF.Exp)
            ones1 = wp.tile([128, 1], F32, tag="ones1")
            nc.vector.memset(ones1, 1.0)
            zpad = wp.tile([128, PADW], F32, tag="zpad")
            nc.vector.memset(zpad, 0.0)

            # conv as PE matmuls with diagonal weights, built on DVE from cst
            # (happens during the input-DMA window: depends only on cst+ident)
            diags = []
            for t_i in range(8):
                dgs = []
                for tap in range(9):
                    d = wp.tile([128, 128], F32R, tag=f"dg{t_i}_{tap}",
                                name=f"dg{t_i}_{tap}")
                    nc.vector.tensor_scalar_mul(
                        d[:, :], ident[:, :],
                        cst[:, 4 + 9 * t_i + tap: 5 + 9 * t_i + tap])
                    dgs.append(d)
                diags.append(dgs)

            kT2 = wp.tile([128, M], F32R, tag="kT2")
            qT2 = wp.tile([128, M], F32R, tag="qT2")
            vaug = [wp.tile([128, HD + 1], F32R, tag=f"vaug{i}", name=f"vaug{i}")
                    for i in range(16)]
            attnT = [wp.tile([128, 256], F32R, tag=f"attnT{i}", name=f"attnT{i}")
                     for i in range(4)]

            # ------------- phase 1: kv proj + conv (+ q proj wedged in) -------------
            # PE streams are in-order: emit kv tiles 0,4 first (their inputs
            # arrive first), then the q projection (its inputs land while
            # kv 0/4 compute), then the remaining kv tiles.
            with tc.tile_pool(name="kvps", bufs=2, space="PSUM") as kvps, \
                 tc.tile_pool(name="cvps", bufs=2, space="PSUM") as cvps, \
                 tc.tile_pool(name="vtps", bufs=1, space="PSUM") as vtps, \
                 tc.tile_pool(name="qps", bufs=1, space="PSUM") as qps:

                def conv_tile(t_i):
                    """kv proj ch-tile t_i -> conv+bias output cacc [128, 256]."""
                    kvp = kvps.tile([128, L], F32, tag="kvp")
                    for n in range(2):
                        for k in range(4):
                            nc.tensor.matmul(kvp[:, n * 512:(n + 1) * 512],
                                             wkv[:, k, t_i * 128:(t_i + 1) * 128],
                                             xT[:, k, n * 512:(n + 1) * 512],
                                             start=(k == 0), stop=(k == 3))
                    # copy into zero-padded 33x33 spatial layout (ACT)
                    kvsb = kvsbp.tile([128, NPAD], F32R, tag="kvsb")
                    pad_col = bass.AP(tensor=kvsb.tensor, offset=kvsb.offset + 32,
                                      ap=[kvsb.ap[0], [PADW, PADW]])
                    nc.vector.tensor_copy(pad_col, zpad[:, :])
                    nc.vector.tensor_copy(kvsb[:, PADW * 32: PADW * 32 + 32],
                                          zpad[:, 0:32])
                    dst = bass.AP(tensor=kvsb.tensor, offset=kvsb.offset,
                                  ap=[kvsb.ap[0], [PADW, 32], [1, 32]])
                    nc.scalar.copy(dst, kvp[:, :].rearrange("p (a b) -> p a b",
                                                            b=32))
                    # 9 conv taps as diag matmuls accumulating in PSUM
                    cvp = cvps.tile([128, NS], F32, tag="cvp")
                    for tap in range(9):
                        dy, dx = tap // 3, tap % 3
                        win = bass.AP(tensor=kvsb.tensor,
                                      offset=kvsb.offset + PADW * dy + dx,
                                      ap=[kvsb.ap[0], [2 * PADW, KH], [2, KW]])
                        nc.tensor.matmul(cvp[:, :], diags[t_i][tap][:, :], win,
                                         start=(tap == 0), stop=(tap == 8))
                    # bias plane add + PSUM->SBUF (DVE)
                    cacc = caccp.tile([128, NS], F32, tag="cacc")
                    nc.vector.scalar_tensor_tensor(
                        cacc[:, :], cvp[:, :], 1.0, bpl[:, t_i, :],
                        op0=ALU.mult, op1=ALU.add)
                    return cacc

                def k_tile(t_i):
                    cacc = conv_tile(t_i)
                    for gi in range(2):
                        g = 2 * t_i + gi
                        for half in range(2):
                            nc.gpsimd.tensor_copy(
                                kT2[half * 64:half * 64 + 64,
                                    g * 256:(g + 1) * 256],
                                cacc[gi * 64:gi * 64 + 64, :])

                def v_tile(t_i):
                    vacc = conv_tile(4 + t_i)
                    for gi in range(2):
                        g = 2 * t_i + gi
                        for sh in range(2):
                            vt = vtps.tile([128, HD], F32, tag="vt")
                            nc.tensor.transpose(
                                vt[:, :],
                                vacc[gi * 64:gi * 64 + 64,
                                     sh * 128:(sh + 1) * 128],
                                ident[gi * 64:gi * 64 + 64,
                                      gi * 64:gi * 64 + 64])
                            mt = g * 2 + sh
                            nc.vector.tensor_copy(vaug[mt][:, 0:HD], vt[:, :])
                            nc.vector.tensor_copy(vaug[mt][:, HD:HD + 1],
                                                  ones1[:, :])

                def q_proj():
                    for t_i in range(4):
                        qp = qps.tile([128, 256], F32, tag="qp")
                        for k in range(4):
                            nc.tensor.matmul(qp[:, :],
                                             wq[:, k, t_i * 128:(t_i + 1) * 128],
                                             xTc[:, k, :],
                                             start=(k == 0), stop=(k == 3))
                        for gi in range(2):          # head 2t+gi
                            h = 2 * t_i + gi
                            for half in range(2):
                                nc.vector.tensor_scalar_add(
                                    qT2[half * 64:half * 64 + 64,
                                        h * 256:(h + 1) * 256],
                                    qp[gi * 64:gi * 64 + 64, :],
                                    cst[gi * 64:gi * 64 + 64, t_i:t_i + 1])

                k_tile(0)
                v_tile(0)
                q_proj()
                for t_i in range(1, 4):
                    k_tile(t_i)
                    v_tile(t_i)

            # ---------------- phase 2: attention ----------------
            with tc.tile_pool(name="sps", bufs=2, space="PSUM") as sps, \
                 tc.tile_pool(name="avps", bufs=2, space="PSUM") as avps:
                qv = qT2.rearrange("p (h l) -> p h l", l=256)
                for lh in range(2):
                    avt = avps.tile([HD + 1, 1024], F32, tag="avt")
                    for mt in range(16):
                        half = mt % 2        # alternate PE row groups
                        st = sps.tile([128, 1024], F32, tag="st")
                        for n in range(2):   # 4 heads per N=512 chunk
                            rhs = qv[half * 64:half * 64 + 64,
                                     4 * n:4 * n + 4,
                                     lh * 128:lh * 128 + 128]
                            nc.tensor.matmul(
                                st[:, n * 512:(n + 1) * 512],
                                kT2[half * 64:half * 64 + 64,
                                    mt * 128:(mt + 1) * 128],
                                rhs, start=True, stop=True,
                                tile_position=(half * 64, 0))
                        ex = expp.tile([128, 1024], F32R, tag="ex")
                        nc.scalar.activation(ex[:, :], st[:, :], AF.Exp,
                                             scale=float(SCALE))
                        for n in range(2):
                            nc.tensor.matmul(
                                avt[:, n * 512:(n + 1) * 512],
                                vaug[mt][:, :],
                                ex[:, n * 512:(n + 1) * 512],
                                start=(mt == 0), stop=(mt == 15))
                    # normalization for this l-half (z lives on partition 0);
                    # recip/broadcast split in halves so the stt chain overlaps
                    zrec = zrbp.tile([1, 1024], F32, tag="zrec")
                    zrb = zrbp.tile([64, 1024], F32, tag="zrb")
                    for hf in range(2):
                        sl = slice(hf * 512, hf * 512 + 512)
                        nc.vector.reciprocal(zrec[:, sl], avt[HD:HD + 1, sl])
                        nc.gpsimd.partition_broadcast(zrb[:, sl],
                                                      zrec[0:1, sl],
                                                      channels=64)
                    for h in range(NH):
                        nc.vector.scalar_tensor_tensor(
                            attnT[h // 2][(h % 2) * 64:(h % 2) * 64 + 64,
                                          lh * 128:lh * 128 + 128],
                            avt[0:HD, h * 128:(h + 1) * 128], 1.0,
                            zrb[:, h * 128:(h + 1) * 128],
                            op0=ALU.mult, op1=ALU.mult)

            # ---------------- phase 3: output projection ----------------
            with tc.tile_pool(name="yps", bufs=2, space="PSUM") as yps:
                ysb = expp.tile([128, 4, 256], F32, tag="ysb")
                for m in range(4):
                    yp = yps.tile([128, 256], F32, tag="yp")
                    for k in range(4):
                        nc.tensor.matmul(yp[:, :],
                                         wo[:, k, m * 128:(m + 1) * 128],
                                         attnT[k][:, :],
                                         start=(k == 0), stop=(k == 3))
                    nc.vector.tensor_scalar_add(ysb[:, m, :], yp[:, :],
                                                cst[:, 76 + m:77 + m])
                    nc.sync.dma_start(out=y_d[m * 128:(m + 1) * 128, :],
                                      in_=ysb[:, m, :])

    nc.finalize()
    return nc


def _get_program():
    if "nc" not in _NC_CACHE:
        _NC_CACHE["nc"] = _build_program()
    return _NC_CACHE["nc"]


def _host_prep(x, wq, bq, wkv, bkv, dw_kernel, dw_bias, wo, bo):
    """Build the 8 per-core input maps."""
    x = np.ascontiguousarray(np.asarray(x, np.float32))
    wq_r = _round_f32r(np.asarray(wq, np.float32))
    wkv_r = _round_f32r(np.asarray(wkv, np.float32))
    wo_r = _round_f32r(np.asarray(wo, np.float32))
    bq = np.asarray(bq, np.float32)
    bkv = np.asarray(bkv, np.float32)
    dw_bias = np.asarray(dw_bias, np.float32)
    bo = np.asarray(bo, np.float32)
    dww = np.asarray(dw_kernel, np.float32).reshape(9, CH).T.copy()  # [1024, 9]

    # bias plane: dw_bias + bkv * sum(valid taps), SAME padding aware
    oy = np.arange(KH)
    valid_y = (2 * oy[:, None] + np.arange(3)[None, :]) < H      # [16, 3]
    valid_x = valid_y.copy()
    wsum = np.zeros((CH, KH, KW), np.float32)
    for tap in range(9):
        dy, dx = tap // 3, tap % 3
        m2 = np.outer(valid_y[:, dy], valid_x[:, dx]).astype(np.float32)
        wsum += dww[:, tap][:, None, None] * m2[None, :, :]
    bpl = (dw_bias[:, None] + bkv[:, None] * wsum.reshape(CH, NS)).astype(np.float32)

    cst = np.zeros((128, 80), np.float32)
    cst[:, 0:4] = bq.reshape(4, 128).T
    for t_i in range(8):
        cst[:, 4 + 9 * t_i: 13 + 9 * t_i] = dww[t_i * 128:(t_i + 1) * 128, :]
    cst[:, 76:80] = bo.reshape(4, 128).T

    in_maps = []
    for c in range(8):
        b, j = c // 4, c % 4
        xT = _round_f32r(x[b].reshape(L, DIM).T)
        in_maps.append({
            "xT": np.ascontiguousarray(xT),
            "xTc": np.ascontiguousarray(xT[:, j * 256:(j + 1) * 256]),
            "wkv": wkv_r, "wq": wq_r, "wo": wo_r,
            "bpl": bpl, "cst": cst,
        })
    return in_maps


def kernel(**inputs) -> np.ndarray:
    nc = _get_program()
    in_maps = _host_prep(**inputs)
    res = run_bass_kernel_spmd(nc, in_maps, core_ids=list(range(8)))
    out = np.zeros((B, H, W, DIM), np.float32)
    flat = out.reshape(B, L, DIM)
    for c in range(8):
        b, j = c // 4, c % 4
        flat[b, j * 256:(j + 1) * 256, :] = res.results[c]["y"].T
    return out



# revision 43
# speedup vs baseline: 1.0426x; 1.0426x over previous
"""MobileMQA Trainium2 kernel (8 NeuronCores, SPMD).

Reference computation (per batch b of 2):
  q  = x @ wq + bq                         [1024 tok, 512]
  kv = x @ wkv + bkv                       [1024 tok, 1024]
  kv = depthwise3x3_s2_same(kv) + dw_bias  [256 sp, 1024]
  k, v = split(kv)  -> reshape to shared-KV length M=2048 (channel fold)
  attn = softmax(q @ k^T * 0.125); out = attn @ v
  y = out @ wo + bo

Sharding: core c handles batch b=c//4, query chunk j=c%4 (256 tokens).
KV path (proj+conv) is replicated across the 4 cores of a batch (MQA).

Design notes (cost-model driven):
- All matmul moving operands are bf16 (1.0 cycles/row at any size; fp32r
  pays 4x below 256 rows). PSUM accumulation stays fp32.
- attn@V is computed with exp-scores as the STATIONARY operand and V as
  the moving one: out[l, d] per head accumulates over 16 m-tiles at 64
  moving rows each (16.4k PE cycles vs 32.8k the other way around).
  Softmax denominators come from an extra ones-column matmul per head.
- Depthwise conv runs on PE as 9 diagonal-weight matmuls per 128-channel
  tile, using per-tap valid-rectangle access patterns (no zero-padding
  pass). Diagonal weight matrices are prebuilt on the host.
- All inputs are packed into one bf16 + one f32 DRAM tensor, DMA'd in a
  handful of large chunks ordered by first use (HWDGE issue costs ~650ns
  per dma_start, so few large DMAs beat many small ones).
- Attention is software-pipelined into the kv/conv phase: score matmuls
  for m-tile group t interleave with projection/conv matmuls of group
  t+1, keeping ACT (exp) busy from ~8us onward.
"""
import os
import sys

for _p in ("/opt/trn_rl_repo", "/opt/trn_rl_repo/concourse"):
    if _p not in sys.path:
        sys.path.insert(0, _p)

_TRUNC = int(os.environ.get("KTRUNC", "9"))

import numpy as np
import ml_dtypes

import concourse.bass as bass
import concourse.mybir as mybir
import concourse.tile as tile
from concourse import bacc
from concourse.bass_utils import run_bass_kernel_spmd
from concourse.masks import make_identity

F32 = mybir.dt.float32
F32R = mybir.dt.float32r
BF16 = mybir.dt.bfloat16
AF = mybir.ActivationFunctionType
ALU = mybir.AluOpType
BFNP = ml_dtypes.bfloat16

DIM = 512
NH = 8
HD = 64
B, H, W = 2, 32, 32
L = H * W            # 1024 tokens per batch
KH = KW = 16
NS = KH * KW         # 256 conv-output spatial positions
M = NS * NH          # 2048 shared-KV positions
CH = 2 * DIM         # 1024 kv channels
SCALE = HD ** -0.5   # 0.125

# ---- f32r mega-tensor column layout (segments ordered by first use) ----
# ch-tile processing order: k0 v0 k1 v1 k2 v2 k3 v3 -> c = 0,4,1,5,2,6,3,7
_CORD = [0, 4, 1, 5, 2, 6, 3, 7]
OFF_WKV = {0: 0, 4: 512, 1: 10496, 5: 11008, 2: 13824, 6: 14336,
           3: 17152, 7: 17664}  # per ch-tile c: [4k, 128] = 512 cols
OFF_XT = {0: 1024, 1: 4224}     # [4k, 512] = 2048 per half
OFF_DIAG = {0: 3072, 4: 6272, 1: 11520, 5: 12672, 2: 14848, 6: 16000,
            3: 18176, 7: 19328}  # per ch-tile c: [9, 128] = 1152 cols
OFF_XTC = 7424         # [4k, 256] = 1024
OFF_WQ = 8448          # [4t, 4k, 128] = 2048
NR = 20480

# f32 tensor: cst (bq 4, bo 4, 8 pad), bias planes per ch-tile
OFF_BPL = {c: 16 + i * 256 for i, c in enumerate(_CORD)}
NF32 = 16 + 8 * 256

# bf16 tensor: wo only
OFF_WO = 0             # [4m, 4k, 128] = 2048
NBF = 2048

# conv taps in emission order (all full-rectangle over a zero-padded 33x33
# input layout, SAME padding)
_TAPS = [(0, 0), (0, 1), (0, 2), (1, 0), (1, 1), (1, 2), (2, 0), (2, 1), (2, 2)]
PADW = 33
NPAD = PADW * PADW   # 1089

_NC_CACHE = {}


def _build_program():
    nc = bacc.Bacc(None)

    bigr_d = nc.dram_tensor("bigr", [128, NR], F32R, kind="ExternalInput")
    bigb_d = nc.dram_tensor("bigb", [128, NBF], BF16, kind="ExternalInput")
    bigf_d = nc.dram_tensor("bigf", [128, NF32], F32, kind="ExternalInput")
    y_d = nc.dram_tensor("y", [DIM, 256], F32, kind="ExternalOutput")

    with tile.TileContext(nc) as tc:
        with tc.tile_pool(name="wp", bufs=1) as wp, \
             tc.tile_pool(name="kvsbp", bufs=2) as kvsbp, \
             tc.tile_pool(name="vsbp", bufs=2) as vsbp, \
             tc.tile_pool(name="expp", bufs=6) as expp:

            bigr = wp.tile([128, NR], F32R, tag="bigr")
            bigb = wp.tile([128, NBF], BF16, tag="bigb")
            bigf = wp.tile([128, NF32], F32, tag="bigf")

            def dma_r(lo, hi):
                nc.sync.dma_start(out=bigr[:, lo:hi], in_=bigr_d[:, lo:hi])

            def dma_b(lo, hi):
                nc.sync.dma_start(out=bigb[:, lo:hi], in_=bigb_d[:, lo:hi])

            def dma_f(lo, hi):
                nc.sync.dma_start(out=bigf[:, lo:hi], in_=bigf_d[:, lo:hi])

            dma_r(0, 1024)          # wkv c0, c4
            dma_r(1024, 3072)       # xT n0
            dma_f(0, 272)           # cst + bpl k0
            dma_r(3072, 4224)       # diag c0
            dma_r(4224, 6272)       # xT n1
            dma_r(6272, 7424)       # diag c4
            dma_f(272, 528)         # bpl v0
            dma_r(7424, 10496)      # xTc + wq
            dma_r(10496, 12672)     # wkv c1,c5 + diag c1
            dma_f(528, NF32)        # bpl rest
            dma_r(12672, 13824)     # diag c5
            dma_r(13824, 16000)     # wkv c2,c6 + diag c2
            dma_r(16000, 17152)     # diag c6
            dma_r(17152, 19328)     # wkv c3,c7 + diag c3
            dma_r(19328, NR)        # diag c7
            dma_b(0, NBF)           # wo

            identf = wp.tile([128, 128], F32, tag="identf")
            make_identity(nc, identf)
            identb = wp.tile([128, 128], BF16, tag="identb")
            nc.vector.tensor_copy(identb[:, :], identf[:, :])
            onesf = wp.tile([128, 1], F32, tag="onesf")
            nc.vector.memset(onesf, 1.0)
            ones1 = wp.tile([128, 1], BF16, tag="ones1")
            nc.vector.tensor_copy(ones1[:, :], onesf[:, :])
            zpad = wp.tile([128, PADW], F32, tag="zpad")
            nc.vector.memset(zpad, 0.0)
            # preload the exp ACT table during the DMA window
            warm = wp.tile([1, 1], F32, tag="warm")
            nc.vector.memset(warm, 0.0)
            nc.scalar.activation(warm[:, :], warm[:, :], AF.Exp)

            kT2 = wp.tile([64, M], F32R, tag="kT2")
            qT2 = wp.tile([64, M], F32R, tag="qT2")
            vaug = wp.tile([128, 16 * HD], BF16, tag="vaug")
            attn_sb = wp.tile([128, 2, 512], BF16, tag="attn_sb")
            attnT = wp.tile([128, 4, 256], BF16, tag="attnT")
            zr = wp.tile([128, 16], F32, tag="zr")
            ysb = wp.tile([128, 4, 256], F32, tag="ysb")

            def wkv_l(c, k):
                o = OFF_WKV[c] + k * 128
                return bigr[:, o:o + 128]

            def xt_r(n, k):
                o = OFF_XT[n] + k * 512
                return bigr[:, o:o + 512]

            def diag_l(c, j):
                o = OFF_DIAG[c] + j * 128
                return bigr[:, o:o + 128]

            def bpl_v(c):
                o = OFF_BPL[c]
                return bigf[:, o:o + 256]

            with tc.tile_pool(name="stp", bufs=2, space="PSUM") as stp, \
                 tc.tile_pool(name="avpp", bufs=1, space="PSUM") as avpp, \
                 tc.tile_pool(name="zpp", bufs=1, space="PSUM") as zpp:

                avp = [avpp.tile([128, 512], F32, tag=f"avp{l}",
                                 name=f"avp{l}") for l in range(2)]
                zp = zpp.tile([128, 16], F32, tag="zp")

                # ---------- kv proj + conv closures (PSUM pool passed in) ----------
                def new_kvsb(name):
                    """Zero-padded 33x33 conv-input layout; the SAME-pad
                    column (32) and bottom row (32) are zeroed on Pool."""
                    kvsb = kvsbp.tile([128, NPAD], F32R, tag="kvsb", name=name)
                    kb = kvsb[:, :]
                    pad_col = bass.AP(tensor=kb.tensor, offset=kb.offset + 32,
                                      ap=[kb.ap[0], [PADW, PADW]])
                    nc.vector.tensor_copy(pad_col, zpad[:, :])
                    nc.vector.tensor_copy(kvsb[:, PADW * 32:PADW * 32 + 32],
                                          zpad[:, 0:32])
                    return kvsb

                def kv_chunk(scr, c, n, kvsb):
                    kvp = scr.tile([128, 512], F32, tag="scr", name="kvp")
                    for k in range(4):
                        nc.tensor.matmul(kvp[:, :], wkv_l(c, k), xt_r(n, k),
                                         start=(k == 0), stop=(k == 3))
                    # 512 tokens = 16 padded rows of 32
                    kb = kvsb[:, :]
                    dst = bass.AP(tensor=kb.tensor,
                                  offset=kb.offset + n * 16 * PADW,
                                  ap=[kb.ap[0], [PADW, 16], [1, 32]])
                    nc.vector.tensor_copy(dst, kvp[:, :])

                def conv_taps(scr, c, kvsb, taps, cvp=None):
                    if cvp is None:
                        cvp = scr.tile([128, 512], F32, tag="scr", name="cvp")
                    kb = kvsb[:, :]
                    for dy, dx in taps:
                        win = bass.AP(tensor=kb.tensor,
                                      offset=kb.offset + PADW * dy + dx,
                                      ap=[kb.ap[0], [2 * PADW, KH], [2, KW]])
                        nc.tensor.matmul(cvp[:, 0:256], diag_l(c, 3 * dy + dx),
                                         win,
                                         start=((dy, dx) == _TAPS[0]),
                                         stop=((dy, dx) == _TAPS[-1]))
                    return cvp

                def k_finish(t, cvp):
                    # conv out + bias plane -> kT2 (shared K head, bf16)
                    for gi in range(2):
                        g = 2 * t + gi
                        nc.vector.scalar_tensor_tensor(
                            kT2[:, g * 256:(g + 1) * 256],
                            cvp[gi * 64:(gi + 1) * 64, 0:256], 1.0,
                            bpl_v(t)[gi * 64:(gi + 1) * 64, :],
                            op0=ALU.mult, op1=ALU.add)

                def v_finish(t, cvp, vtp):
                    # transpose [128 ch, 128 s] -> [128 s, 128 ch]; the two
                    # 64-wide ch-groups then scatter to their m-tiles
                    c = 4 + t
                    vsb = vsbp.tile([128, 256], BF16, tag="vsb")
                    nc.vector.scalar_tensor_tensor(
                        vsb[:, :], cvp[:, 0:256], 1.0, bpl_v(c)[:, :],
                        op0=ALU.mult, op1=ALU.add)
                    for sh in range(2):
                        vt = vtp.tile([128, 128], BF16, tag="vt")
                        nc.tensor.matmul(vt[:, :],
                                         vsb[:, sh * 128:(sh + 1) * 128],
                                         identb[:, :], is_transpose=True)
                        av = vaug[:, :]
                        dst = bass.AP(tensor=av.tensor,
                                      offset=av.offset + t * 256 + sh * 64,
                                      ap=[av.ap[0], [128, 2], [1, 64]])
                        nc.vector.tensor_copy(dst, vt[:, :])

                def qproj_one(scr, t):
                    qp = scr.tile([128, 512], F32, tag="scr", name="qp")
                    for k in range(4):
                        nc.tensor.matmul(
                            qp[:, 0:256],
                            bigr[:, OFF_WQ + t * 512 + k * 128:
                                 OFF_WQ + t * 512 + (k + 1) * 128],
                            bigr[:, OFF_XTC + k * 256:OFF_XTC + (k + 1) * 256],
                            start=(k == 0), stop=(k == 3))
                    for gi in range(2):
                        h = 2 * t + gi
                        nc.vector.tensor_scalar_add(
                            qT2[:, h * 256:(h + 1) * 256],
                            qp[gi * 64:(gi + 1) * 64, 0:256],
                            bigf[gi * 64:(gi + 1) * 64, t:t + 1])

                def kvconv_thunks(scr, vtp, t):
                    """10 thunks: k-tile t then v-tile t."""
                    st8 = {}

                    def kt_chunk(n):
                        if "k" not in st8:
                            st8["k"] = new_kvsb("kvsbk")
                        kv_chunk(scr, t, n, st8["k"])

                    def kt_conv(lo):
                        if "kc" not in st8:
                            st8["kc"] = conv_taps(scr, t, st8["k"], _TAPS[:4])
                        else:
                            conv_taps(scr, t, st8["k"], _TAPS[4:], st8["kc"])

                    def vt_chunk(n):
                        if "v" not in st8:
                            st8["v"] = new_kvsb("kvsbv")
                        kv_chunk(scr, 4 + t, n, st8["v"])

                    def vt_conv(lo):
                        if "vc" not in st8:
                            st8["vc"] = conv_taps(scr, 4 + t, st8["v"], _TAPS[:4])
                        else:
                            conv_taps(scr, 4 + t, st8["v"], _TAPS[4:], st8["vc"])

                    return [
                        lambda: kt_chunk(0),
                        lambda: kt_chunk(1),
                        lambda: kt_conv(0),
                        lambda: kt_conv(4),
                        lambda: k_finish(t, st8["kc"]),
                        lambda: vt_chunk(0),
                        lambda: vt_chunk(1),
                        lambda: vt_conv(0),
                        lambda: vt_conv(4),
                        lambda: (v_finish(t, st8["vc"], vtp)),
                    ]

                # ---------- attention chunk emission ----------
                qv = qT2[:, :].rearrange("p (h l) -> p h l", l=256)

                def emit_S(c):
                    mt, lh, hg = c
                    st = stp.tile([128, 512], F32, tag="st")
                    rhs = qv[:, hg * 4:(hg + 1) * 4, lh * 128:(lh + 1) * 128]
                    nc.tensor.matmul(st[:, :], kT2[:, mt * 128:(mt + 1) * 128],
                                     rhs, start=True, stop=True)
                    ex = expp.tile([128, 512], BF16, tag="ex")
                    nc.scalar.activation(ex[:, :], st[:, :], AF.Exp,
                                         scale=float(SCALE))
                    return ex

                def emit_AV(c, ex):
                    # One PSUM accumulation group per bank: the first matmul
                    # start=True lazily zeroes the whole 2KB region; each
                    # head's first write consumes its share of the zeroing.
                    mt, lh, hg = c
                    for hi in range(4):
                        h = hg * 4 + hi
                        exh = ex[:, hi * 128:(hi + 1) * 128]
                        nc.tensor.matmul(avp[lh][:, h * 64:(h + 1) * 64],
                                         exh, vaug[:, mt * 64:(mt + 1) * 64],
                                         start=(mt == 0 and h == 0),
                                         stop=(mt == 15 and h == 7))
                        nc.tensor.matmul(zp[:, lh * 8 + h:lh * 8 + h + 1],
                                         exh, ones1[:, :],
                                         start=(mt == 0 and lh == 0 and h == 0),
                                         stop=(mt == 15 and lh == 1 and h == 7))

                # ---------- tail closures ----------
                def norm_lh(lh):
                    for h in range(NH):
                        nc.vector.tensor_scalar_mul(
                            attn_sb[:, lh, h * 64:(h + 1) * 64],
                            avp[lh][:, h * 64:(h + 1) * 64],
                            zr[:, lh * 8 + h:lh * 8 + h + 1])

                def tr_lh(lh, trp):
                    for kk in range(4):
                        trt = trp.tile([128, 128], BF16, tag="trp")
                        nc.tensor.matmul(
                            trt[:, :],
                            attn_sb[:, lh, kk * 128:(kk + 1) * 128],
                            identb[:, :], is_transpose=True)
                        nc.vector.tensor_copy(
                            attnT[:, kk, lh * 128:(lh + 1) * 128], trt[:, :])

                def y_m(m, lh, ypp, dma_engine=None):
                    yp = ypp.tile([128, 256], F32, tag="yp")
                    for k in range(4):
                        nc.tensor.matmul(
                            yp[:, 0:128],
                            bigb[:, m * 512 + k * 128:m * 512 + (k + 1) * 128],
                            attnT[:, k, lh * 128:(lh + 1) * 128],
                            start=(k == 0), stop=(k == 3))
                    nc.vector.tensor_scalar_add(
                        ysb[:, m, lh * 128:(lh + 1) * 128], yp[:, 0:128],
                        bigf[:, 4 + m:5 + m])
                    if dma_engine is not None:
                        dma_engine.dma_start(out=y_d[m * 128:(m + 1) * 128, :],
                                             in_=ysb[:, m, :])

                # ---------- chunk schedule ----------
                # chunks: blocks tau=0..3; tau<3 mt-major, tau=3 lh-major
                chunks = []
                for tau in range(3):
                    for mt in range(4 * tau, 4 * tau + 4):
                        for lh in range(2):
                            for hg in range(2):
                                chunks.append((mt, lh, hg))
                for lh in range(2):
                    for mt in range(12, 16):
                        for hg in range(2):
                            chunks.append((mt, lh, hg))

                LAG = 2
                pend = []

                def run_chunks(lo, hi, fills):
                    for i in range(lo, hi):
                        for th in fills.get(i, ()):
                            th()
                        pend.append((chunks[i], emit_S(chunks[i])))
                        if len(pend) > LAG:
                            c, ex = pend.pop(0)
                            emit_AV(c, ex)

                def flush_pend():
                    while pend:
                        c, ex = pend.pop(0)
                        emit_AV(c, ex)

                FILL_SLOTS = [0, 1, 3, 4, 6, 8, 9, 11, 12, 14]

                with tc.tile_pool(name="scr", bufs=2, space="PSUM") as scr, \
                     tc.tile_pool(name="vtp", bufs=1, space="PSUM") as vtp:
                    # prologue: tile 0 kv+conv, q projection
                    if _TRUNC >= 1:
                        nth = int(os.environ.get("KTHUNKS", "10"))
                        for th in kvconv_thunks(scr, vtp, 0)[:nth]:
                            th()
                    if _TRUNC >= 2:
                        for t in range(4):
                            qproj_one(scr, t)
                    # blocks 0-2 with kvconv fills for t+1
                    if _TRUNC >= 3:
                        for tau in range(3):
                            ths = kvconv_thunks(scr, vtp, tau + 1) \
                                if tau < 3 else []
                            fills = {tau * 16 + s: [ths[j]]
                                     for j, s in enumerate(FILL_SLOTS)} \
                                if ths else {}
                            run_chunks(tau * 16, tau * 16 + 16, fills)

                with tc.tile_pool(name="trp", bufs=2, space="PSUM") as trp, \
                     tc.tile_pool(name="ypp", bufs=1, space="PSUM") as ypp:
                    if _TRUNC >= 4:
                        run_chunks(48, 64, {})
                        flush_pend()
                    if _TRUNC >= 5:
                        nc.vector.reciprocal(zr[:, :], zp[:, :])
                        norm_lh(0)
                        tr_lh(0, trp)
                        norm_lh(1)
                        y_m(0, 0, ypp)
                        y_m(1, 0, ypp)
                        tr_lh(1, trp)
                        y_m(2, 0, ypp)
                        y_m(3, 0, ypp)
                        y_m(0, 1, ypp, nc.sync)
                        y_m(1, 1, ypp, nc.scalar)
                        y_m(2, 1, ypp, nc.sync)
                        y_m(3, 1, ypp, nc.scalar)
                    else:
                        nc.vector.memset(ysb, 0.0)
                        for m in range(4):
                            nc.sync.dma_start(out=y_d[m * 128:(m + 1) * 128, :],
                                              in_=ysb[:, m, :])

    nc.finalize()
    return nc


def _get_program():
    if "nc" not in _NC_CACHE:
        _NC_CACHE["nc"] = _build_program()
    return _NC_CACHE["nc"]


def _host_prep(x, wq, bq, wkv, bkv, dw_kernel, dw_bias, wo, bo):
    """Build the 8 per-core input maps (bigb bf16 + bigf f32)."""
    x = np.ascontiguousarray(np.asarray(x, np.float32))
    wq = np.asarray(wq, np.float32)
    wkv = np.asarray(wkv, np.float32)
    wo = np.asarray(wo, np.float32)
    bq = np.asarray(bq, np.float32)
    bkv = np.asarray(bkv, np.float32)
    dw_bias = np.asarray(dw_bias, np.float32)
    bo = np.asarray(bo, np.float32)
    dww = np.asarray(dw_kernel, np.float32).reshape(9, CH).T.copy()  # [1024, 9]

    # bias plane: dw_bias + bkv * sum(valid taps), SAME padding aware
    oy = np.arange(KH)
    valid_y = (2 * oy[:, None] + np.arange(3)[None, :]) < H      # [16, 3]
    valid_x = valid_y.copy()
    wsum = np.zeros((CH, KH, KW), np.float32)
    for tap in range(9):
        dy, dx = tap // 3, tap % 3
        m2 = np.outer(valid_y[:, dy], valid_x[:, dx]).astype(np.float32)
        wsum += dww[:, tap][:, None, None] * m2[None, :, :]
    bpl = (dw_bias[:, None] + bkv[:, None] * wsum.reshape(CH, NS)).astype(np.float32)

    # ---- shared f32r template (np.float32; PE rounds internally) ----
    tmpl = np.zeros((128, NR), np.float32)
    ar = np.arange(128)
    for c in range(8):
        blk = wkv.reshape(4, 128, 8, 128)[:, :, c, :]          # [k, p, cc]
        tmpl[:, OFF_WKV[c]:OFF_WKV[c] + 512] = \
            blk.transpose(1, 0, 2).reshape(128, 512)
        dblk = np.zeros((128, 9, 128), np.float32)
        for j in range(9):
            dblk[ar, j, ar] = dww[c * 128 + ar, j]
        tmpl[:, OFF_DIAG[c]:OFF_DIAG[c] + 1152] = dblk.reshape(128, 1152)
    wqb = wq.reshape(4, 128, 4, 128)                           # [k, p, t, cc]
    for t in range(4):
        tmpl[:, OFF_WQ + t * 512:OFF_WQ + (t + 1) * 512] = \
            wqb[:, :, t, :].transpose(1, 0, 2).reshape(128, 512)

    bigb = np.zeros((128, NBF), BFNP)
    wob = wo.reshape(4, 128, 4, 128)
    for m in range(4):
        bigb[:, m * 512:(m + 1) * 512] = \
            wob[:, :, m, :].transpose(1, 0, 2).reshape(128, 512).astype(BFNP)

    bigf = np.zeros((128, NF32), np.float32)
    bigf[:, 0:4] = bq.reshape(4, 128).T
    bigf[:, 4:8] = bo.reshape(4, 128).T
    for c in range(8):
        bigf[:, OFF_BPL[c]:OFF_BPL[c] + 256] = bpl[c * 128:(c + 1) * 128, :]

    in_maps = []
    for core in range(8):
        b, j = core // 4, core % 4
        xtb = x[b].reshape(L, DIM).T                            # [512, 1024]
        br = tmpl.copy()
        xa = xtb.reshape(4, 128, 2, 512)                        # [k, p, n, t']
        for n in range(2):
            br[:, OFF_XT[n]:OFF_XT[n] + 2048] = \
                xa[:, :, n, :].transpose(1, 0, 2).reshape(128, 2048)
        xc = xtb[:, j * 256:(j + 1) * 256].reshape(4, 128, 256)
        br[:, OFF_XTC:OFF_XTC + 1024] = \
            xc.transpose(1, 0, 2).reshape(128, 1024)
        in_maps.append({"bigr": br, "bigb": bigb, "bigf": bigf})
    return in_maps


def kernel(**inputs) -> np.ndarray:
    nc = _get_program()
    in_maps = _host_prep(**inputs)
    res = run_bass_kernel_spmd(nc, in_maps, core_ids=list(range(8)))
    out = np.zeros((B, H, W, DIM), np.float32)
    flat = out.reshape(B, L, DIM)
    for c in range(8):
        b, j = c // 4, c % 4
        flat[b, j * 256:(j + 1) * 256, :] = res.results[c]["y"].T
    return out
